# revision 6
# baseline (speedup 1.0000x reference)
"""GQA attention kernel for Trainium2, 8 NeuronCores — low-DMA revision.

Sharding: TP-4 (kv-head pairs) x DP-2 (batch). Core c = b*4 + g handles
batch b, q-heads 8g..8g+7, kv-heads 2g..2g+1. Each core computes a partial
(D, S) output (its heads' contribution through wo); host sums the 4 partials
per batch.

vs the previous revision, all HBM traffic moves through a handful of large
DMAs whose DRAM layouts are host-swizzled so every transfer is one
contiguous run per SBUF partition (128 descriptors per DMA):
  - x arrives as xP[p, qc, kt, s]: one DMA per chunk (32KB/partition run).
  - wq/wo are streamed per chunk in 8 pieces each; wk/wv resident.
  - RoPE uses de-interleaved q/k feature rows (evens on partitions 0-63,
    odds on 64-127, via host-permuted wq/wk rows) so the pair rotation is
    two half-height DVE muls with cross-partition operands — no SBUF-SBUF
    swap DMAs at all.
  - output staged per (chunk, head-group) and written with one DMA each.

Everything else (scores^T orientation, ones-matmul softmax denominator,
normalization folded into the PSUM->SBUF copy) matches the prior revision.
"""

import sys

if "/opt/trn_rl_repo" not in sys.path:
    sys.path.insert(0, "/opt/trn_rl_repo")

import math
import os

import ml_dtypes
import numpy as np

BF16 = ml_dtypes.bfloat16

B = 2
S = 2048
D = 4096
H = 32
KVH = 8
HD = 128
P = 128
TPG = 4                 # tensor-parallel groups (per batch)
LQH = H // TPG          # 8 local q heads
LKVH = KVH // TPG       # 2 local kv heads
QF = LQH * HD           # 1024 local q features
KF = LKVH * HD          # 256 local kv features
CHUNK = 512
NCHUNK = S // CHUNK     # 4
KT = D // P             # 32 contraction tiles for projections
SCALE = 1.0 / math.sqrt(HD)

_BUILT = None
LAST_EXEC_TIME_NS = None


def _build_program():
    import concourse.bass as bass  # noqa: F401
    import concourse.tile as tile
    from concourse import bacc, mybir

    nc = bacc.Bacc("TRN2", target_bir_lowering=False, debug=False,
                   num_devices=8)
    f32 = mybir.dt.float32
    b16 = mybir.dt.bfloat16

    xP = nc.dram_tensor("xP", [P, NCHUNK, KT, CHUNK], b16,
                        kind="ExternalInput").ap()
    wqP = nc.dram_tensor("wqP", [P, KT, QF], b16, kind="ExternalInput").ap()
    woQ = nc.dram_tensor("woQ", [P, 8, LQH, CHUNK], b16,
                         kind="ExternalInput").ap()
    # all resident constants in one tensor/DMA:
    #   [wk (KT*KF) | wv (KT*KF) | cos (S) | sin (S) | mask (4*CHUNK)]
    NCONST = 2 * KT * KF + 2 * S + 4 * CHUNK
    constP = nc.dram_tensor("constP", [P, NCONST], b16,
                            kind="ExternalInput").ap()
    # outP[p, mt, s] = partial_out[mt*128+p, s]
    outP = nc.dram_tensor("outP", [P, KT, S], b16,
                          kind="ExternalOutput").ap()

    Exp = mybir.ActivationFunctionType.Exp

    with tile.TileContext(nc) as tc:
        with (
            tc.tile_pool(name="consts", bufs=1) as consts,
            tc.tile_pool(name="persist", bufs=1) as persist,
            tc.tile_pool(name="qpool", bufs=1) as qpool,
            tc.tile_pool(name="stream", bufs=1) as stream,
            tc.tile_pool(name="work", bufs=1) as work,
            tc.tile_pool(name="ps", bufs=1, space="PSUM") as ps,
        ):
            # ---- constants: one DMA, AP views ----
            const_sb = consts.tile([P, NCONST], b16, name="const_sb")
            ones_f = consts.tile([P, P], f32, name="ones_f")
            ones_b = consts.tile([P, P], b16, name="ones_b")

            nc.sync.dma_start(const_sb, constP)
            KW = KT * KF
            wk_sb = const_sb[:, 0:KW].rearrange("p (k f) -> p k f", k=KT)
            wv_sb = const_sb[:, KW:2 * KW].rearrange("p (k f) -> p k f", k=KT)
            cos_sb = const_sb[:, 2 * KW:2 * KW + S]
            sin_sb = const_sb[:, 2 * KW + S:2 * KW + 2 * S]
            mask_sb = const_sb[:, 2 * KW + 2 * S:].rearrange(
                "p (r f) -> p r f", r=4)

            # ---- persistent K^T (roped, de-interleaved rows) and V ----
            kT_sb = persist.tile([P, LKVH, S], b16, name="kT_sb")
            v_sb = persist.tile([P, S // P, KF], b16, name="v_sb")

            nc.vector.memset(ones_f, 1.0)
            nc.vector.tensor_copy(out=ones_b, in_=ones_f)

            def psum_tile(nm):
                return ps.tile([P, CHUNK], f32, tag="ps", bufs=8, name=nm)

            def rope(dst, src_psum, tsl, nm):
                """dst = rope(src_psum), de-interleaved feature layout.

                Partition p<64 holds even feature 2p ("a"), p>=64 holds odd
                feature 2(p-64)+1 ("b").  out_a = a*cos - b*sin,
                out_b = a*sin + b*cos.  cos_sb duplicates cos on both
                halves; sin_sb holds -sin on the top half, +sin on the
                bottom, so out[p] = src[p]*cos_sb[p] + src[p^64]*sin_sb[p].
                """
                tmp = work.tile([P, CHUNK], f32, tag="rtmp", bufs=2,
                                name=f"rt{nm}")
                nc.vector.tensor_mul(out=tmp[0:64, :], in0=src_psum[64:P, :],
                                     in1=sin_sb[0:64, tsl])
                nc.vector.tensor_mul(out=tmp[64:P, :], in0=src_psum[0:64, :],
                                     in1=sin_sb[64:P, tsl])
                nc.vector.tensor_mul(out=dst, in0=src_psum,
                                     in1=cos_sb[:, tsl])
                nc.vector.tensor_add(out=dst, in0=dst, in1=tmp)

            # attention output for the whole sequence; consumed by the
            # final wo phase so wo streams through SBUF exactly once.
            attnT_sb = qpool.tile([P, LQH, S], b16, name="attnT")

            for qc in range(NCHUNK):
                tsl = slice(qc * CHUNK, (qc + 1) * CHUNK)

                # ---- x for this chunk: one 32KB/partition DMA ----
                x_t = stream.tile([P, KT, CHUNK], b16, tag="xc", bufs=1,
                                  name=f"x{qc}")
                nc.sync.dma_start(x_t, xP[:, qc])

                # ======== pass A: Q projection ========
                qT_sb = qpool.tile([P, LQH, CHUNK], b16, tag="qT", bufs=1,
                                   name=f"qT{qc}")
                qps = [psum_tile(f"qp{qc}_{m}") for m in range(LQH)]
                for pc in range(2):
                    wq_t = stream.tile([P, 16, QF], b16, tag="wq", bufs=1,
                                       name=f"wq{qc}_{pc}")
                    nc.sync.dma_start(wq_t, wqP[:, pc * 16:(pc + 1) * 16, :])
                    for ki in range(16):
                        kt = pc * 16 + ki
                        for m in range(LQH):
                            nc.tensor.matmul(
                                qps[m],
                                wq_t[:, ki, m * P:(m + 1) * P],
                                x_t[:, kt, :],
                                start=(kt == 0), stop=(kt == KT - 1))
                for m in range(LQH):
                    rope(qT_sb[:, m, :], qps[m], tsl, f"q{qc}_{m}")

                # ======== pass B: K, V projections ========
                kps = [psum_tile(f"kp{qc}_{j}") for j in range(LKVH)]
                vps = [psum_tile(f"vp{qc}_{st}") for st in range(4)]
                for kt in range(KT):
                    for j in range(LKVH):
                        nc.tensor.matmul(
                            kps[j],
                            wk_sb[:, kt, j * P:(j + 1) * P],
                            x_t[:, kt, :],
                            start=(kt == 0), stop=(kt == KT - 1))
                    for st in range(4):
                        nc.tensor.matmul(
                            vps[st][:, :KF],
                            x_t[:, kt, st * P:(st + 1) * P],
                            wv_sb[:, kt, :],
                            start=(kt == 0), stop=(kt == KT - 1))
                for j in range(LKVH):
                    rope(kT_sb[:, j, tsl], kps[j], tsl, f"k{qc}_{j}")
                for st in range(4):
                    nc.vector.tensor_copy(
                        out=v_sb[:, qc * 4 + st, :], in_=vps[st][:, :KF])

                # ======== attention for this chunk ========
                NT = 4 * qc + 4
                LOOK = 3
                flat = [(h, kt) for h in range(LQH) for kt in range(NT)]
                exq = {}
                opvs = {}
                dpss = {}

                def emit_scores(h, kt):
                    kv = h // 4
                    sps = psum_tile(f"sp{qc}_{h}_{kt}")
                    nc.tensor.matmul(
                        sps,
                        kT_sb[:, kv, kt * P:(kt + 1) * P],
                        qT_sb[:, h, :], start=True, stop=True)
                    ex = work.tile([P, CHUNK], b16, tag="exp", bufs=5,
                                   name=f"ex{qc}_{h}_{kt}")
                    nc.scalar.activation(out=ex, in_=sps, func=Exp,
                                         scale=SCALE)
                    r = kt - 4 * qc
                    if r >= 0:
                        nc.vector.tensor_mul(out=ex, in0=ex,
                                             in1=mask_sb[:, r, :])
                    exq[(h, kt)] = ex

                for h, kt in flat[:LOOK]:
                    emit_scores(h, kt)
                for i, (h, kt) in enumerate(flat):
                    if i + LOOK < len(flat):
                        emit_scores(*flat[i + LOOK])
                    kv = h // 4
                    if kt == 0:
                        opvs[h] = psum_tile(f"ov{qc}_{h}")
                        dpss[h] = psum_tile(f"dp{qc}_{h}")
                    ex = exq.pop((h, kt))
                    nc.tensor.matmul(
                        opvs[h],
                        v_sb[:, kt, kv * P:(kv + 1) * P],
                        ex,
                        start=(kt == 0), stop=(kt == NT - 1))
                    # denominator accumulates on PE: broadcast partition sum
                    nc.tensor.matmul(
                        dpss[h], ones_b, ex,
                        start=(kt == 0), stop=(kt == NT - 1))
                    if kt == NT - 1:
                        drec = work.tile([P, CHUNK], f32, tag="drec", bufs=1,
                                         name=f"dr{qc}_{h}")
                        nc.vector.reciprocal(out=drec, in_=dpss.pop(h))
                        nc.vector.tensor_mul(out=attnT_sb[:, h, tsl],
                                             in0=opvs.pop(h), in1=drec)

            # ======== output projection (all chunks; wo streamed once) ====
            for mtg in range(8):
                wo_t = stream.tile([P, LQH, CHUNK], b16, tag="wo",
                                   bufs=2, name=f"wo{mtg}")
                nc.sync.dma_start(wo_t, woQ[:, mtg])
                osb = work.tile([P, 4, S], b16, tag="osb", bufs=1,
                                name=f"ou{mtg}")
                for qc in range(NCHUNK):
                    tsl = slice(qc * CHUNK, (qc + 1) * CHUNK)
                    ops = [psum_tile(f"op{qc}_{mtg}_{mi}") for mi in range(4)]
                    for kf in range(LQH):
                        for mi in range(4):
                            nc.tensor.matmul(
                                ops[mi],
                                wo_t[:, kf, mi * P:(mi + 1) * P],
                                attnT_sb[:, kf, tsl],
                                start=(kf == 0), stop=(kf == LQH - 1))
                    for mi in range(4):
                        nc.vector.tensor_copy(out=osb[:, mi, tsl],
                                              in_=ops[mi])
                nc.sync.dma_start(
                    outP[:, mtg * 4:(mtg + 1) * 4, :], osb)

    nc.compile()
    return nc


def _deint(n_heads):
    """Row permutation de-interleaving head_dim pairs within each head."""
    idx = []
    for h in range(n_heads):
        base = h * HD
        idx.extend(base + 2 * i for i in range(HD // 2))
        idx.extend(base + 2 * i + 1 for i in range(HD // 2))
    return np.asarray(idx)


def _host_inputs(x, cos, sin, wq, wk, wv, wo):
    """Per-core input dicts. Core c = b*TPG + g."""
    cosT = np.ascontiguousarray(cos.T.astype(np.float32))   # (64, S)
    sinT = np.ascontiguousarray(sin.T.astype(np.float32))
    cosP = np.concatenate([cosT, cosT], axis=0).astype(BF16)  # (128, S)
    sinP = np.concatenate([-sinT, sinT], axis=0).astype(BF16)

    pp = np.arange(P)[:, None]
    ff = np.arange(CHUNK)[None, :]
    maskP = np.empty((P, 4, CHUNK), BF16)
    for r in range(4):
        maskP[:, r, :] = (r * P + pp <= ff).astype(BF16)

    qperm = _deint(LQH)
    kperm = _deint(LKVH)

    in_maps = []
    for c in range(8):
        b, g = divmod(c, TPG)
        qsl = slice(g * QF, (g + 1) * QF)
        ksl = slice(g * KF, (g + 1) * KF)

        # xP[p, qc, kt, s'] = x[b, qc*512+s', kt*128+p]
        xT = x[b].T.astype(BF16)                            # (D, S)
        xPc = np.ascontiguousarray(
            xT.reshape(KT, P, NCHUNK, CHUNK).transpose(1, 2, 0, 3))

        # wqP[p, kt, qf] = wq[qsl][qperm].T[kt*128+p, qf]
        wqT = wq[qsl][qperm].T.astype(BF16)                 # (D, QF)
        wqPc = np.ascontiguousarray(
            wqT.reshape(KT, P, QF).transpose(1, 0, 2))
        wkT = wk[ksl][kperm].T.astype(BF16)                 # (D, KF)
        wkPc = wkT.reshape(KT, P, KF).transpose(1, 0, 2)
        wvT = wv[ksl].T.astype(BF16)
        wvPc = wvT.reshape(KT, P, KF).transpose(1, 0, 2)
        constPc = np.concatenate([
            wkPc.reshape(P, KT * KF),
            wvPc.reshape(P, KT * KF),
            cosP, sinP,
            maskP.reshape(P, 4 * CHUNK),
        ], axis=1)

        # woQ[p, mtg, kf, dd] = wo[:, qsl].T[kf*128+p, mtg*512+dd]
        woT = wo[:, qsl].T.astype(BF16)                     # (QF, D)
        woQc = np.ascontiguousarray(
            woT.reshape(LQH, P, 8, CHUNK).transpose(1, 2, 0, 3))

        in_maps.append({
            "xP": xPc,
            "wqP": wqPc,
            "woQ": woQc,
            "constP": np.ascontiguousarray(constPc),
        })
    return in_maps


def kernel(x, cos, sin, wq, wk, wv, wo):
    global _BUILT
    from concourse.bass_utils import run_bass_kernel_spmd

    x = np.asarray(x, np.float32)
    cos = np.asarray(cos, np.float32)
    sin = np.asarray(sin, np.float32)
    wq = np.asarray(wq, np.float32)
    wk = np.asarray(wk, np.float32)
    wv = np.asarray(wv, np.float32)
    wo = np.asarray(wo, np.float32)

    if _BUILT is None:
        _BUILT = _build_program()
    nc = _BUILT

    in_maps = _host_inputs(x, cos, sin, wq, wk, wv, wo)
    trace = os.environ.get("KERNEL_TRACE") == "1"
    try:
        res = run_bass_kernel_spmd(nc, in_maps, core_ids=list(range(8)),
                                   trace=trace)
    except Exception:
        if not trace:
            raise
        # profiling unavailable in this environment; run without it
        res = run_bass_kernel_spmd(nc, in_maps, core_ids=list(range(8)))
    global LAST_EXEC_TIME_NS
    LAST_EXEC_TIME_NS = getattr(res, "exec_time_ns", None)
    if trace and LAST_EXEC_TIME_NS is not None:
        print(f"HW exec time: {LAST_EXEC_TIME_NS} ns")

    out = np.zeros((B, S, D), np.float32)
    for c in range(8):
        b = c // TPG
        # outP[p, mt, s] -> partial (S, D)
        o = res.results[c]["outP"].astype(np.float32)
        o = o.transpose(1, 0, 2).reshape(D, S)
        out[b] += o.T
    return out


# revision 13
# speedup vs baseline: 1.4542x; 1.4542x over previous
"""GQA attention kernel for Trainium2, 8 NeuronCores.

Sharding: TP-4 (kv-head pairs) x DP-2 (batch). Core c = b*4 + g handles
batch b, q-heads 8g..8g+7, kv-heads 2g..2g+1. Each core computes a partial
(D, S) output (its heads' contribution through wo); host sums the 4 partials
per batch.

All HBM traffic moves through ~29 large DMAs whose DRAM layouts are
host-swizzled so every transfer is one contiguous run per SBUF partition
(128 descriptors per DMA, ~3.7k descriptors total):
  - x arrives as xP[p, qc, kt, s]: one DMA per chunk (32KB/partition run).
  - wq streams per chunk in two 32KB/partition head-half pieces through a
    single buffer; the K-projection matmuls are emitted between the halves
    so the in-order PE queue covers the second piece's DMA.
  - wk/wv/cos/sin/mask are resident, loaded by ONE packed DMA (constP).
  - wo streams once, in the single output-projection phase at the end,
    which consumes the attention output for the whole sequence (attnT_sb).
  - output staged per head-group in SBUF and written with one DMA each.

RoPE uses de-interleaved q/k feature rows (evens on partitions 0-63, odds
on 64-127, via host-permuted wq/wk rows) so the pair rotation is two
half-height DVE muls with cross-partition operands — no SBUF-SBUF swap
DMAs. Scores are computed transposed (key, query) so PV needs no
transpose; the softmax denominator accumulates on the PE via a ones
matmul (broadcast form) and the normalization folds into the PSUM->SBUF
copy.
"""

import sys

if "/opt/trn_rl_repo" not in sys.path:
    sys.path.insert(0, "/opt/trn_rl_repo")

import math
import os

import ml_dtypes
import numpy as np

BF16 = ml_dtypes.bfloat16

B = 2
S = 2048
D = 4096
H = 32
KVH = 8
HD = 128
P = 128
TPG = 4                 # tensor-parallel groups (per batch)
LQH = H // TPG          # 8 local q heads
LKVH = KVH // TPG       # 2 local kv heads
QF = LQH * HD           # 1024 local q features
KF = LKVH * HD          # 256 local kv features
CHUNK = 512
NCHUNK = S // CHUNK     # 4
KT = D // P             # 32 contraction tiles for projections
SCALE = 1.0 / math.sqrt(HD)

_BUILT = None
LAST_EXEC_TIME_NS = None


def _build_program():
    import concourse.bass as bass  # noqa: F401
    import concourse.tile as tile
    from concourse import bacc, mybir

    nc = bacc.Bacc("TRN2", target_bir_lowering=False, debug=False,
                   num_devices=8)
    f32 = mybir.dt.float32
    b16 = mybir.dt.bfloat16

    xP = nc.dram_tensor("xP", [P, NCHUNK, KT, CHUNK], b16,
                        kind="ExternalInput").ap()
    # wqP[p, mh, kt, qf'] = wq_perm.T[kt*128+p, mh*512+qf']  (m-halves)
    wqP = nc.dram_tensor("wqP", [P, 2, KT, QF // 2], b16,
                         kind="ExternalInput").ap()
    woQ = nc.dram_tensor("woQ", [P, 8, LQH, CHUNK], b16,
                         kind="ExternalInput").ap()
    # all resident constants in one tensor/DMA:
    #   [wk (KT*KF) | wv (KT*KF) | cos (S) | sin (S) | mask (4*CHUNK)]
    NCONST = 2 * KT * KF + 2 * S + 4 * CHUNK
    constP = nc.dram_tensor("constP", [P, NCONST], b16,
                            kind="ExternalInput").ap()
    # outP[p, mt, s] = partial_out[mt*128+p, s]
    outP = nc.dram_tensor("outP", [P, KT, S], b16,
                          kind="ExternalOutput").ap()

    Exp = mybir.ActivationFunctionType.Exp

    with tile.TileContext(nc) as tc:
        with (
            tc.tile_pool(name="consts", bufs=1) as consts,
            tc.tile_pool(name="persist", bufs=1) as persist,
            tc.tile_pool(name="qpool", bufs=1) as qpool,
            tc.tile_pool(name="stream", bufs=1) as stream,
            tc.tile_pool(name="work", bufs=1) as work,
            tc.tile_pool(name="ps", bufs=1, space="PSUM") as ps,
        ):
            # ---- constants: one DMA, AP views ----
            const_sb = consts.tile([P, NCONST], b16, name="const_sb")
            ones_f = consts.tile([P, P], f32, name="ones_f")
            ones_b = consts.tile([P, P], b16, name="ones_b")

            nc.sync.dma_start(const_sb, constP)
            KW = KT * KF
            wk_sb = const_sb[:, 0:KW].rearrange("p (k f) -> p k f", k=KT)
            wv_sb = const_sb[:, KW:2 * KW].rearrange("p (k f) -> p k f", k=KT)
            cos_sb = const_sb[:, 2 * KW:2 * KW + S]
            sin_sb = const_sb[:, 2 * KW + S:2 * KW + 2 * S]
            mask_sb = const_sb[:, 2 * KW + 2 * S:].rearrange(
                "p (r f) -> p r f", r=4)

            # ---- persistent K^T (roped, de-interleaved rows) and V ----
            kT_sb = persist.tile([P, LKVH, S], b16, name="kT_sb")
            v_sb = persist.tile([P, S // P, KF], b16, name="v_sb")

            nc.vector.memset(ones_f, 1.0)
            nc.vector.tensor_copy(out=ones_b, in_=ones_f)

            def psum_tile(nm):
                return ps.tile([P, CHUNK], f32, tag="ps", bufs=8, name=nm)

            def rope(dst, src_psum, tsl, nm):
                """dst = rope(src_psum), de-interleaved feature layout.

                Partition p<64 holds even feature 2p ("a"), p>=64 holds odd
                feature 2(p-64)+1 ("b").  out_a = a*cos - b*sin,
                out_b = a*sin + b*cos.  cos_sb duplicates cos on both
                halves; sin_sb holds -sin on the top half, +sin on the
                bottom, so out[p] = src[p]*cos_sb[p] + src[p^64]*sin_sb[p].
                """
                tmp = work.tile([P, CHUNK], f32, tag="rtmp", bufs=2,
                                name=f"rt{nm}")
                nc.vector.tensor_mul(out=tmp[0:64, :], in0=src_psum[64:P, :],
                                     in1=sin_sb[0:64, tsl])
                nc.vector.tensor_mul(out=tmp[64:P, :], in0=src_psum[0:64, :],
                                     in1=sin_sb[64:P, tsl])
                nc.vector.tensor_mul(out=dst, in0=src_psum,
                                     in1=cos_sb[:, tsl])
                nc.vector.tensor_add(out=dst, in0=dst, in1=tmp)

            # attention output for the whole sequence; consumed by the
            # final wo phase so wo streams through SBUF exactly once.
            attnT_sb = qpool.tile([P, LQH, S], b16, name="attnT")

            for qc in range(NCHUNK):
                tsl = slice(qc * CHUNK, (qc + 1) * CHUNK)

                # ---- x for this chunk: one 32KB/partition DMA ----
                x_t = stream.tile([P, KT, CHUNK], b16, tag="xc", bufs=1,
                                  name=f"x{qc}")
                nc.sync.dma_start(x_t, xP[:, qc])

                # ======== pass A+B: projections ========
                # Q streams wq in two head-half pieces (single buffer); the
                # K matmuls are emitted between the halves so the PE covers
                # the second piece's DMA, and V after the second half.
                qT_sb = qpool.tile([P, LQH, CHUNK], b16, tag="qT", bufs=1,
                                   name=f"qT{qc}")
                wq_t0 = stream.tile([P, KT, QF // 2], b16, tag="wq", bufs=1,
                                    name=f"wq{qc}_0")
                nc.sync.dma_start(wq_t0, wqP[:, 0])
                qps0 = [psum_tile(f"qp{qc}_{mi}") for mi in range(4)]
                for kt in range(KT):
                    for mi in range(4):
                        nc.tensor.matmul(
                            qps0[mi],
                            wq_t0[:, kt, mi * P:(mi + 1) * P],
                            x_t[:, kt, :],
                            start=(kt == 0), stop=(kt == KT - 1))

                kps = [psum_tile(f"kp{qc}_{j}") for j in range(LKVH)]
                for kt in range(KT):
                    for j in range(LKVH):
                        nc.tensor.matmul(
                            kps[j],
                            wk_sb[:, kt, j * P:(j + 1) * P],
                            x_t[:, kt, :],
                            start=(kt == 0), stop=(kt == KT - 1))
                for mi in range(4):
                    rope(qT_sb[:, mi, :], qps0[mi], tsl, f"q{qc}_{mi}")

                wq_t1 = stream.tile([P, KT, QF // 2], b16, tag="wq", bufs=1,
                                    name=f"wq{qc}_1")
                nc.sync.dma_start(wq_t1, wqP[:, 1])
                qps1 = [psum_tile(f"qp{qc}_{4 + mi}") for mi in range(4)]
                for kt in range(KT):
                    for mi in range(4):
                        nc.tensor.matmul(
                            qps1[mi],
                            wq_t1[:, kt, mi * P:(mi + 1) * P],
                            x_t[:, kt, :],
                            start=(kt == 0), stop=(kt == KT - 1))
                for j in range(LKVH):
                    rope(kT_sb[:, j, tsl], kps[j], tsl, f"k{qc}_{j}")

                vps = [psum_tile(f"vp{qc}_{st}") for st in range(4)]
                for kt in range(KT):
                    for st in range(4):
                        nc.tensor.matmul(
                            vps[st][:, :KF],
                            x_t[:, kt, st * P:(st + 1) * P],
                            wv_sb[:, kt, :],
                            start=(kt == 0), stop=(kt == KT - 1))
                for mi in range(4):
                    rope(qT_sb[:, 4 + mi, :], qps1[mi], tsl, f"q{qc}_{4 + mi}")
                for st in range(4):
                    nc.vector.tensor_copy(
                        out=v_sb[:, qc * 4 + st, :], in_=vps[st][:, :KF])

                # ======== attention for this chunk ========
                NT = 4 * qc + 4
                LOOK = 3
                flat = [(h, kt) for h in range(LQH) for kt in range(NT)]
                exq = {}
                opvs = {}
                dpss = {}

                def emit_scores(h, kt):
                    kv = h // 4
                    sps = psum_tile(f"sp{qc}_{h}_{kt}")
                    nc.tensor.matmul(
                        sps,
                        kT_sb[:, kv, kt * P:(kt + 1) * P],
                        qT_sb[:, h, :], start=True, stop=True)
                    ex = work.tile([P, CHUNK], b16, tag="exp", bufs=5,
                                   name=f"ex{qc}_{h}_{kt}")
                    nc.scalar.activation(out=ex, in_=sps, func=Exp,
                                         scale=SCALE)
                    r = kt - 4 * qc
                    if r >= 0:
                        nc.vector.tensor_mul(out=ex, in0=ex,
                                             in1=mask_sb[:, r, :])
                    exq[(h, kt)] = ex

                for h, kt in flat[:LOOK]:
                    emit_scores(h, kt)
                for i, (h, kt) in enumerate(flat):
                    if i + LOOK < len(flat):
                        emit_scores(*flat[i + LOOK])
                    kv = h // 4
                    if kt == 0:
                        opvs[h] = psum_tile(f"ov{qc}_{h}")
                        dpss[h] = psum_tile(f"dp{qc}_{h}")
                    ex = exq.pop((h, kt))
                    nc.tensor.matmul(
                        opvs[h],
                        v_sb[:, kt, kv * P:(kv + 1) * P],
                        ex,
                        start=(kt == 0), stop=(kt == NT - 1))
                    # denominator accumulates on PE: broadcast partition sum
                    nc.tensor.matmul(
                        dpss[h], ones_b, ex,
                        start=(kt == 0), stop=(kt == NT - 1))
                    if kt == NT - 1:
                        drec = work.tile([P, CHUNK], f32, tag="drec", bufs=1,
                                         name=f"dr{qc}_{h}")
                        nc.vector.reciprocal(out=drec, in_=dpss.pop(h))
                        nc.vector.tensor_mul(out=attnT_sb[:, h, tsl],
                                             in0=opvs.pop(h), in1=drec)

            # ======== output projection (all chunks; wo streamed once) ====
            for mtg in range(8):
                wo_t = stream.tile([P, LQH, CHUNK], b16, tag="wo",
                                   bufs=2, name=f"wo{mtg}")
                nc.sync.dma_start(wo_t, woQ[:, mtg])
                osb = work.tile([P, 4, S], b16, tag="osb", bufs=1,
                                name=f"ou{mtg}")
                for qc in range(NCHUNK):
                    tsl = slice(qc * CHUNK, (qc + 1) * CHUNK)
                    ops = [psum_tile(f"op{qc}_{mtg}_{mi}") for mi in range(4)]
                    for kf in range(LQH):
                        for mi in range(4):
                            nc.tensor.matmul(
                                ops[mi],
                                wo_t[:, kf, mi * P:(mi + 1) * P],
                                attnT_sb[:, kf, tsl],
                                start=(kf == 0), stop=(kf == LQH - 1))
                    for mi in range(4):
                        nc.vector.tensor_copy(out=osb[:, mi, tsl],
                                              in_=ops[mi])
                nc.sync.dma_start(
                    outP[:, mtg * 4:(mtg + 1) * 4, :], osb)

    nc.compile()
    return nc


def _deint(n_heads):
    """Row permutation de-interleaving head_dim pairs within each head."""
    idx = []
    for h in range(n_heads):
        base = h * HD
        idx.extend(base + 2 * i for i in range(HD // 2))
        idx.extend(base + 2 * i + 1 for i in range(HD // 2))
    return np.asarray(idx)


def _host_inputs(x, cos, sin, wq, wk, wv, wo):
    """Per-core input dicts. Core c = b*TPG + g."""
    cosT = np.ascontiguousarray(cos.T.astype(np.float32))   # (64, S)
    sinT = np.ascontiguousarray(sin.T.astype(np.float32))
    cosP = np.concatenate([cosT, cosT], axis=0).astype(BF16)  # (128, S)
    sinP = np.concatenate([-sinT, sinT], axis=0).astype(BF16)

    pp = np.arange(P)[:, None]
    ff = np.arange(CHUNK)[None, :]
    maskP = np.empty((P, 4, CHUNK), BF16)
    for r in range(4):
        maskP[:, r, :] = (r * P + pp <= ff).astype(BF16)

    qperm = _deint(LQH)
    kperm = _deint(LKVH)

    in_maps = []
    for c in range(8):
        b, g = divmod(c, TPG)
        qsl = slice(g * QF, (g + 1) * QF)
        ksl = slice(g * KF, (g + 1) * KF)

        # xP[p, qc, kt, s'] = x[b, qc*512+s', kt*128+p]
        xT = x[b].T.astype(BF16)                            # (D, S)
        xPc = np.ascontiguousarray(
            xT.reshape(KT, P, NCHUNK, CHUNK).transpose(1, 2, 0, 3))

        # wqP[p, mh, kt, qf'] = wq[qsl][qperm].T[kt*128+p, mh*512+qf']
        wqT = wq[qsl][qperm].T.astype(BF16)                 # (D, QF)
        wqPc = np.ascontiguousarray(
            wqT.reshape(KT, P, 2, QF // 2).transpose(1, 2, 0, 3))
        wkT = wk[ksl][kperm].T.astype(BF16)                 # (D, KF)
        wkPc = wkT.reshape(KT, P, KF).transpose(1, 0, 2)
        wvT = wv[ksl].T.astype(BF16)
        wvPc = wvT.reshape(KT, P, KF).transpose(1, 0, 2)
        constPc = np.concatenate([
            wkPc.reshape(P, KT * KF),
            wvPc.reshape(P, KT * KF),
            cosP, sinP,
            maskP.reshape(P, 4 * CHUNK),
        ], axis=1)

        # woQ[p, mtg, kf, dd] = wo[:, qsl].T[kf*128+p, mtg*512+dd]
        woT = wo[:, qsl].T.astype(BF16)                     # (QF, D)
        woQc = np.ascontiguousarray(
            woT.reshape(LQH, P, 8, CHUNK).transpose(1, 2, 0, 3))

        in_maps.append({
            "xP": xPc,
            "wqP": wqPc,
            "woQ": woQc,
            "constP": np.ascontiguousarray(constPc),
        })
    return in_maps


def kernel(x, cos, sin, wq, wk, wv, wo):
    global _BUILT
    from concourse.bass_utils import run_bass_kernel_spmd

    x = np.asarray(x, np.float32)
    cos = np.asarray(cos, np.float32)
    sin = np.asarray(sin, np.float32)
    wq = np.asarray(wq, np.float32)
    wk = np.asarray(wk, np.float32)
    wv = np.asarray(wv, np.float32)
    wo = np.asarray(wo, np.float32)

    if _BUILT is None:
        _BUILT = _build_program()
    nc = _BUILT

    in_maps = _host_inputs(x, cos, sin, wq, wk, wv, wo)
    trace = os.environ.get("KERNEL_TRACE") == "1"
    try:
        res = run_bass_kernel_spmd(nc, in_maps, core_ids=list(range(8)),
                                   trace=trace)
    except Exception:
        if not trace:
            raise
        # profiling unavailable in this environment; run without it
        res = run_bass_kernel_spmd(nc, in_maps, core_ids=list(range(8)))
    global LAST_EXEC_TIME_NS
    LAST_EXEC_TIME_NS = getattr(res, "exec_time_ns", None)
    if trace and LAST_EXEC_TIME_NS is not None:
        print(f"HW exec time: {LAST_EXEC_TIME_NS} ns")

    out = np.zeros((B, S, D), np.float32)
    for c in range(8):
        b = c // TPG
        # outP[p, mt, s] -> partial (S, D)
        o = res.results[c]["outP"].astype(np.float32)
        o = o.transpose(1, 0, 2).reshape(D, S)
        out[b] += o.T
    return out
